# revision 4
# baseline (speedup 1.0000x reference)
"""MLA-style causal self-attention on 8 Trainium2 NeuronCores.

Sharding: tensor-parallel over heads (2 heads/core) for q-decode /
attention / out-proj; phase A (latents = W_qkv^T x^T) is T-sharded:
each core computes only its 256-column slice and the slices are
exchanged with three DRAM AllGathers (kv+rope first, then the two c_q
halves) issued from the gpsimd queue at group milestones so gather
network time overlaps the remaining latent compute. Host sums the 8
row-split out^T partials.

All matmul operands are bf16 (fp32 PSUM accumulate): same PE rate as
fp32r on TRN2 (1 cycle/column at moving>=256) but half the DMA/SBUF
traffic. c_q windows, y^T and W_out live in SBUF; no DRAM scratch
round-trips. Attention is flash-style block-causal with a diagonal
trim (key block r only computed for queries >= 128r) and a single
128x128 triangular mask; softmax denominator via a ones-column matmul
accumulated in PSUM, reciprocal + gpsimd partition-broadcast off the
critical path. Phase D (out-proj) is emitted inside the last head's
loop, one 512-column block after each query chunk completes, so its
dependency-free matmuls fill B/C pipeline bubbles and only the final
block runs after the attention tail.
"""

import math
from contextlib import ExitStack

import numpy as np
import ml_dtypes

import concourse.bass as bass
import concourse.tile as tile
from concourse import bacc, mybir, bass_isa
from concourse.bass_utils import run_bass_kernel_spmd
from concourse.masks import make_identity

F32 = mybir.dt.float32
BF = mybir.dt.bfloat16
AF = mybir.ActivationFunctionType
NPBF = ml_dtypes.bfloat16

T_FULL = 2048
E = 2048
KV = 512
QL = 1024
RH = 64
QKH = KV + RH     # 576
NH = 16
NCORES = 8
HPC = NH // NCORES
SCALE = 1.0 / math.sqrt(float(KV))

P = 128


def _make_rot64(nc, pool):
    rt0 = pool.tile([RH, RH], F32, tag="rt0")
    nc.gpsimd.memset(rt0[:], 0.0)
    nc.gpsimd.affine_select(
        out=rt0[:], in_=rt0[:], compare_op=mybir.AluOpType.not_equal,
        fill=1.0, base=-32, channel_multiplier=1, pattern=[[-1, RH]],
    )
    nc.gpsimd.affine_select(
        out=rt0[:], in_=rt0[:], compare_op=mybir.AluOpType.not_equal,
        fill=1.0, base=32, channel_multiplier=1, pattern=[[-1, RH]],
    )
    rt = pool.tile([RH, RH], BF, tag="rt")
    nc.vector.tensor_copy(rt[:], rt0[:])
    return rt


def build_kernel(T=T_FULL):
    assert T % 512 == 0
    NT512 = T // 512
    NKT = T // P
    EK = E // P
    TS = T // NCORES      # 256

    nc = bacc.Bacc("TRN2", target_bir_lowering=False, debug=False,
                   num_devices=NCORES)

    xTs = nc.dram_tensor("xTs", [E, TS], BF, kind="ExternalInput").ap()
    wqkv = nc.dram_tensor("wqkv", [E, QKH + QL], BF, kind="ExternalInput").ap()
    wqdec = nc.dram_tensor("wqdec", [QL, HPC * QKH], BF, kind="ExternalInput").ap()
    wout = nc.dram_tensor("wout", [HPC * KV, E], BF, kind="ExternalInput").ap()
    cosd = nc.dram_tensor("cosT", [RH, T], F32, kind="ExternalInput").ap()
    sind = nc.dram_tensor("sinT", [RH, T], F32, kind="ExternalInput").ap()
    outT = nc.dram_tensor("outT", [E, T], F32, kind="ExternalOutput").ap()

    xs_r = xTs.rearrange("(ko p) t -> p ko t", p=P)
    wq_r = wqkv.rearrange("(ko p) m -> p ko m", p=P)

    groups = [(i * P, P) for i in range(KV // P)] + [(KV, RH)] + [
        (QKH + i * P, P) for i in range(QL // P)
    ]
    NKVG = KV // P + 1      # groups 0..4 are kv+rope
    NCQ0 = NKVG + 4         # groups 5..8 -> cq half 0

    with tile.TileContext(nc) as tc, ExitStack() as ctx:
        dram = ctx.enter_context(tc.tile_pool(name="dram", bufs=1, space="DRAM"))
        cst = ctx.enter_context(tc.tile_pool(name="cst", bufs=1))
        kvp = ctx.enter_context(tc.tile_pool(name="kvp", bufs=1))
        pp = ctx.enter_context(tc.tile_pool(name="pp", bufs=2, space="PSUM"))
        ppy = ctx.enter_context(tc.tile_pool(name="ppy", bufs=1, space="PSUM"))

        bkv = dram.tile([QKH, TS], BF, tag="bkv")
        bcq0 = dram.tile([QL // 2, TS], BF, tag="bcq0")
        bcq1 = dram.tile([QL // 2, TS], BF, tag="bcq1")
        gkv = dram.tile([NCORES, QKH, TS], BF, tag="gkv",
                        addr_space="Shared")
        gcq0 = dram.tile([NCORES, QL // 2, TS], BF, tag="gcq0",
                         addr_space="Shared")
        gcq1 = dram.tile([NCORES, QL // 2, TS], BF, tag="gcq1",
                         addr_space="Shared")

        # ---- constants (before phase A so gpsimd is free afterwards) ----
        ident0 = cst.tile([P, P], F32, tag="ident0")
        make_identity(nc, ident0[:])
        ident = cst.tile([P, P], BF, tag="ident")
        nc.vector.tensor_copy(ident[:], ident0[:])
        rt = _make_rot64(nc, cst)
        # single triangular mask: mtri[k, q] = 1 iff k <= q (block diagonal)
        mtri = cst.tile([P, P], BF, tag="mtri")
        nc.gpsimd.memset(mtri[:], 1.0)
        nc.gpsimd.affine_select(
            out=mtri[:], in_=mtri[:], compare_op=mybir.AluOpType.is_ge,
            fill=0.0, base=0, channel_multiplier=-1, pattern=[[1, P]],
        )
        ones0 = cst.tile([P, 1], F32, tag="ones0")
        nc.gpsimd.memset(ones0[:], 1.0)
        ones_col = cst.tile([P, 1], BF, tag="ones")
        nc.vector.tensor_copy(ones_col[:], ones0[:])
        # cos/sin tiles declared here; DMAs issued at B/C setup (they are
        # not needed until rope and would delay phase-A weight loads)
        cosT = cst.tile([RH, T], F32, tag="cosT")
        ssinT = cst.tile([RH, T], F32, tag="ssinT")

        ckvT = kvp.tile([P, KV // P, T], BF, tag="ckvT")
        krT = kvp.tile([RH, T], BF, tag="krT")
        krT2 = kvp.tile([RH, T], BF, tag="krT2")
        yT_sb = kvp.tile([P, HPC * KV // P, T], BF, tag="yT_sb")
        wo_sb = kvp.tile([P, HPC * KV // P, E], BF, tag="wo_sb")

        RG = [list(range(NCORES))]

        # ================= Phase A (T-sharded) ============================
        with ExitStack() as actx:
            aw = actx.enter_context(tc.tile_pool(name="aw", bufs=1))
            astp = actx.enter_context(tc.tile_pool(name="astp", bufs=3))

            xs = aw.tile([P, EK, TS], BF, tag="xs")
            nc.scalar.dma_start(xs[:, 0 : EK // 2], xs_r[:, 0 : EK // 2])
            nc.sync.dma_start(xs[:, EK // 2 :], xs_r[:, EK // 2 :])
            # kv-group weights first (they gate the first collective);
            # late cq weights avoid gpsimd so the collectives aren't queued
            # behind transfers
            wqt = [None] * len(groups)
            qeng = [nc.scalar, nc.sync, nc.gpsimd]
            order = list(range(NKVG)) + list(range(NKVG, len(groups)))
            for idx, gi in enumerate(order):
                c0, M = groups[gi]
                eng = (qeng[idx % 3] if gi < NKVG
                       else qeng[idx % 2])
                wa = aw.tile([P, EK, M], BF, tag=f"wqa{gi}", name=f"wqa{gi}")
                eng.dma_start(wa[:], wq_r[:, :, c0 : c0 + M])
                wqt[gi] = wa

            for gi, (c0, M) in enumerate(groups):
                ps = pp.tile([P, TS], F32, tag="mm", name="psA")
                for kc in range(EK):
                    nc.tensor.matmul(
                        ps[:M], wqt[gi][:, kc, :], xs[:, kc, :],
                        start=(kc == 0), stop=(kc == EK - 1),
                    )
                st = astp.tile([P, TS], BF, tag="ast", name="ast")
                nc.vector.tensor_copy(st[:M], ps[:M])
                # bounce writes on gpsimd: they directly precede (and gate)
                # the collectives on that same queue
                if c0 < QKH:
                    nc.gpsimd.dma_start(bkv[c0 : c0 + M, :], st[:M])
                elif c0 < QKH + QL // 2:
                    nc.gpsimd.dma_start(
                        bcq0[c0 - QKH : c0 - QKH + M, :], st[:M])
                else:
                    nc.gpsimd.dma_start(
                        bcq1[c0 - QKH - QL // 2 : c0 - QKH - QL // 2 + M, :],
                        st[:M])
                if gi == NKVG - 1:
                    nc.gpsimd.collective_compute(
                        "AllGather", mybir.AluOpType.bypass,
                        replica_groups=RG, ins=[bkv[:]], outs=[gkv[:]],
                    )
                elif gi == NCQ0 - 1:
                    nc.gpsimd.collective_compute(
                        "AllGather", mybir.AluOpType.bypass,
                        replica_groups=RG, ins=[bcq0[:]], outs=[gcq0[:]],
                    )
            nc.gpsimd.collective_compute(
                "AllGather", mybir.AluOpType.bypass,
                replica_groups=RG, ins=[bcq1[:]], outs=[gcq1[:]],
            )

        # ============ Phases preC + fused B/C =============================
        with ExitStack() as bctx:
            bcp = bctx.enter_context(tc.tile_pool(name="bcp", bufs=1))
            bcs = bctx.enter_context(tc.tile_pool(name="bcs", bufs=2))

            # rope tables + wqd for both heads, prefetched on scalar
            nc.scalar.dma_start(cosT[:], cosd[:])
            nc.scalar.dma_start(ssinT[:], sind[:])
            nc.vector.tensor_scalar_mul(
                ssinT[0 : RH // 2, :], ssinT[0 : RH // 2, :], -1.0
            )
            wqds = []
            for h in range(HPC):
                wqd = bcp.tile([P, QL // P, QKH], BF, tag=f"wqd{h}",
                               name=f"wqd{h}")
                nc.scalar.dma_start(
                    wqd[:],
                    wqdec.rearrange("(ko p) m -> p ko m", p=P)[
                        :, :, h * QKH : (h + 1) * QKH
                    ],
                )
                wqds.append(wqd)

            # unpack kv gather into SBUF residents
            for seg in range(NCORES):
                tsl = slice(seg * TS, (seg + 1) * TS)
                nc.scalar.dma_start(
                    ckvT[:, :, tsl],
                    gkv[seg, 0:KV, :].rearrange("(ko p) t -> p ko t", p=P),
                )
                nc.sync.dma_start(krT[:, tsl], gkv[seg, KV:QKH, :])

            # rope k_r
            for tcc in range(NT512):
                tsl = slice(tcc * 512, (tcc + 1) * 512)
                pr = pp.tile([RH, 512], F32, tag="mm", name="prk")
                nc.tensor.matmul(pr[:], rt[:], krT[:, tsl],
                                 start=True, stop=True)
                nc.vector.tensor_mul(krT2[:, tsl], krT[:, tsl], cosT[:, tsl])
                rot = bcs.tile([RH, 512], F32, tag="rot", name="rotk")
                nc.vector.tensor_mul(rot[:], pr[:], ssinT[:, tsl])
                nc.vector.tensor_add(krT2[:, tsl], krT2[:, tsl], rot[:])

            # v[t, d] via PE transposes of c_kv^T
            v = bcp.tile([P, NKT, KV], BF, tag="v")
            with tc.tile_pool(name="ptr", bufs=2, space="PSUM") as ptr:
                for dc in range(KV // P):
                    for tt in range(NKT):
                        pt = ptr.tile([P, P], BF, tag="tr", name="pt")
                        nc.tensor.transpose(
                            pt[:], ckvT[:, dc, tt * P : (tt + 1) * P], ident[:]
                        )
                        nc.vector.tensor_copy(
                            v[:, tt, dc * P : (dc + 1) * P], pt[:]
                        )

            # c_q windows resident: loaded once, shared by both heads
            cqw = []
            for i4 in range(NT512):
                cqt = bcp.tile([P, QL // P, 2, TS], BF, tag=f"cqw{i4}",
                               name=f"cqw{i4}")
                for s in range(2):
                    nc.scalar.dma_start(
                        cqt[:, 0 : QL // (2 * P), s],
                        gcq0[2 * i4 + s].rearrange("(ko p) t -> p ko t", p=P),
                    )
                    nc.sync.dma_start(
                        cqt[:, QL // (2 * P) : QL // P, s],
                        gcq1[2 * i4 + s].rearrange("(ko p) t -> p ko t", p=P),
                    )
                cqw.append(cqt)

            # W_out resident for phase D, queued behind everything on sync
            wo_r = wout.rearrange("(ko p) e -> p ko e", p=P)
            nc.sync.dma_start(wo_sb[:], wo_r[:])

            pden = bctx.enter_context(
                tc.tile_pool(name="pden", bufs=2, space="PSUM")
            )
            dst = bctx.enter_context(tc.tile_pool(name="dst", bufs=1))

            DK = HPC * KV // P
            wr_eng = [nc.scalar, nc.sync, nc.gpsimd]

            def emit_d_block(tcc):
                # phase-D tile for 512 output columns: interleaved into the
                # last head's loop so its (dependency-free) matmuls fill
                # B/C pipeline bubbles and only the final block runs after
                tsl = slice(tcc * 512, (tcc + 1) * 512)
                for mc in range(E // P):
                    psD = ppy.tile([P, 512], F32, tag=f"y{mc % 4}",
                                   name=f"psD{mc % 4}")
                    for kc in range(DK):
                        nc.tensor.matmul(
                            psD[:], wo_sb[:, kc, mc * P : (mc + 1) * P],
                            yT_sb[:, kc, tsl],
                            start=(kc == 0), stop=(kc == DK - 1),
                        )
                    ost = dst.tile([P, 512], F32, tag=f"ost{mc % 3}",
                                   name="ost")
                    if mc % 2 == 0:
                        nc.vector.tensor_copy(ost[:], psD[:])
                    else:
                        nc.scalar.copy(ost[:], psD[:])
                    wr_eng[mc % 3].dma_start(
                        outT[mc * P : (mc + 1) * P, tsl], ost[:]
                    )

            qgroups = [(KV, RH)] + [(i * P, P) for i in range(KV // P)]
            for h in range(HPC):
                wqd = wqds[h]
                for i4 in range(NT512):
                    qsl = slice(i4 * 512, (i4 + 1) * 512)
                    # ---- B: q^T for queries i4 (SCALE folded in) ----
                    cq = cqw[i4]
                    qTc = [bcs.tile([P, 512], BF, tag=f"qTc{i}",
                                    name=f"qTc{i}") for i in range(KV // P)]
                    qrRaw = bcs.tile([RH, 512], BF, tag="qrRaw", name="qrRaw")
                    qrT = bcs.tile([RH, 512], BF, tag="qrT", name="qrT")
                    for (m0, M) in qgroups:
                        ps = pp.tile([P, 512], F32, tag="mm", name="psB")
                        for kc in range(QL // P):
                            nc.tensor.matmul(
                                ps[:M], wqd[:, kc, m0 : m0 + M],
                                cq[:, kc],
                                start=(kc == 0), stop=(kc == QL // P - 1),
                            )
                        if m0 < KV:
                            nc.vector.tensor_scalar_mul(
                                qTc[m0 // P][:], ps[:], SCALE
                            )
                        else:
                            nc.vector.tensor_scalar_mul(qrRaw[:], ps[:RH], SCALE)
                            pr = pp.tile([RH, 512], F32, tag="mm", name="prq")
                            nc.tensor.matmul(pr[:], rt[:], qrRaw[:],
                                             start=True, stop=True)
                            nc.vector.tensor_mul(qrT[:], qrRaw[:], cosT[:, qsl])
                            rot = bcs.tile([RH, 512], F32, tag="rot",
                                           name="rotq")
                            nc.vector.tensor_mul(rot[:], pr[:], ssinT[:, qsl])
                            nc.vector.tensor_add(qrT[:], qrT[:], rot[:])

                    # ---- C: causal attention for queries i4 ----
                    nj = 4 * i4 + 4
                    psden = pden.tile([1, 512], F32, tag="den", name="psden")
                    psy = [ppy.tile([P, 512], F32, tag=f"y{dc}",
                                    name=f"psy{dc}")
                           for dc in range(KV // P)]
                    for j in range(nj):
                        ksl = slice(j * P, (j + 1) * P)
                        # diagonal trim: key block j only matters for
                        # queries >= 128*r (r = block index within chunk)
                        r = j - 4 * i4
                        qo = P * r if r >= 0 else 0
                        W = 512 - qo
                        ps = pp.tile([P, 512], F32, tag="mm", name="psS")
                        for dc in range(KV // P):
                            nc.tensor.matmul(
                                ps[:, :W], ckvT[:, dc, ksl],
                                qTc[dc][:, qo:],
                                start=(dc == 0), stop=False,
                            )
                        nc.tensor.matmul(
                            ps[:, :W], krT2[:, ksl], qrT[:, qo:],
                            start=False, stop=True,
                        )
                        se = bcs.tile([P, 512], BF, tag="se", bufs=3,
                                      name="se")
                        nc.scalar.activation(se[:, :W], ps[:, :W], AF.Exp)
                        if r >= 0:
                            nc.vector.tensor_mul(
                                se[:, 0:P], se[:, 0:P], mtri[:])
                        nc.tensor.matmul(
                            psden[:, qo:], ones_col[:], se[:, :W],
                            start=(j == 0), stop=(j == nj - 1),
                            skip_group_check=True,
                        )
                        for dc in range(KV // P):
                            nc.tensor.matmul(
                                psy[dc][:, qo:],
                                v[:, j, dc * P : (dc + 1) * P],
                                se[:, :W],
                                start=(j == 0), stop=(j == nj - 1),
                                skip_group_check=True,
                            )
                    deninv = bcs.tile([1, 512], F32, tag="deninv",
                                      name="deninv")
                    nc.vector.reciprocal_approx_fast(out=deninv[:],
                                                     in_=psden[:])
                    denb = bcs.tile([P, 512], F32, tag="denb", name="denb")
                    nc.gpsimd.partition_broadcast(denb[:], deninv[:])
                    for dc in range(KV // P):
                        nc.vector.tensor_mul(
                            yT_sb[:, h * (KV // P) + dc, qsl],
                            psy[dc][:], denb[:],
                        )
                    if h == HPC - 1:
                        emit_d_block(i4)

    nc.compile()
    return nc


_NC_CACHE = {}


def _get_nc(T=T_FULL):
    if T not in _NC_CACHE:
        _NC_CACHE[T] = build_kernel(T)
    return _NC_CACHE[T]


def make_in_maps(x, cos, sin, W_qkv, W_qdec, W_out):
    xT = np.ascontiguousarray(np.asarray(x)[0].T)
    cosT = np.ascontiguousarray(np.asarray(cos).T.astype(np.float32))
    sinT = np.ascontiguousarray(np.asarray(sin).T.astype(np.float32))
    wqkv = np.ascontiguousarray(np.asarray(W_qkv)).astype(NPBF)
    W_qdec = np.asarray(W_qdec)
    W_out = np.asarray(W_out)
    T = xT.shape[1]
    TS = T // NCORES
    in_maps = []
    for c in range(NCORES):
        in_maps.append({
            "xTs": np.ascontiguousarray(
                xT[:, c * TS : (c + 1) * TS]).astype(NPBF),
            "wqkv": wqkv,
            "wqdec": np.ascontiguousarray(
                W_qdec[:, c * HPC * QKH : (c + 1) * HPC * QKH]).astype(NPBF),
            "wout": np.ascontiguousarray(
                W_out[c * HPC * KV : (c + 1) * HPC * KV]).astype(NPBF),
            "cosT": cosT,
            "sinT": sinT,
        })
    return in_maps


def kernel(x, cos, sin, W_qkv, W_qdec, W_out, _trace=False, _tmpdir=None):
    T = np.asarray(x).shape[1]
    nc = _get_nc(T)
    in_maps = make_in_maps(x, cos, sin, W_qkv, W_qdec, W_out)
    res = run_bass_kernel_spmd(
        nc, in_maps, core_ids=list(range(NCORES)),
        trace=_trace, tmpdir=_tmpdir,
    )
    out = np.zeros((E, T), np.float32)
    for r in res.results:
        out += r["outT"]
    kernel.last_results = res
    return np.ascontiguousarray(out.T)[None].astype(np.float32)


# revision 5
# speedup vs baseline: 1.0087x; 1.0087x over previous
"""MLA-style causal self-attention on 8 Trainium2 NeuronCores.

Sharding: tensor-parallel over heads (2 heads/core) for q-decode /
attention / out-proj; phase A (latents = W_qkv^T x^T) is T-sharded:
each core computes only its 256-column slice and the slices are
exchanged with three DRAM AllGathers (kv+rope first, then the two c_q
halves) issued from the gpsimd queue at group milestones so gather
network time overlaps the remaining latent compute. Host sums the 8
row-split out^T partials.

All matmul operands are bf16 (fp32 PSUM accumulate): same PE rate as
fp32r on TRN2 (1 cycle/column at moving>=256) but half the DMA/SBUF
traffic. c_q windows, y^T and W_out live in SBUF; no DRAM scratch
round-trips. Attention is flash-style block-causal with a diagonal
trim (key block r only computed for queries >= 128r) and a single
128x128 triangular mask; softmax denominator via a ones-column matmul
accumulated in PSUM, reciprocal + gpsimd partition-broadcast off the
critical path. Phase D (out-proj) is emitted inside the last head's
loop, one 512-column block after each query chunk completes, so its
dependency-free matmuls fill B/C pipeline bubbles and only the final
block runs after the attention tail.
"""

import math
from contextlib import ExitStack

import numpy as np
import ml_dtypes

import concourse.bass as bass
import concourse.tile as tile
from concourse import bacc, mybir, bass_isa
from concourse.bass_utils import run_bass_kernel_spmd
from concourse.masks import make_identity

F32 = mybir.dt.float32
BF = mybir.dt.bfloat16
AF = mybir.ActivationFunctionType
NPBF = ml_dtypes.bfloat16

T_FULL = 2048
E = 2048
KV = 512
QL = 1024
RH = 64
QKH = KV + RH     # 576
NH = 16
NCORES = 8
HPC = NH // NCORES
SCALE = 1.0 / math.sqrt(float(KV))

P = 128


def _make_rot64(nc, pool):
    rt0 = pool.tile([RH, RH], F32, tag="rt0")
    nc.gpsimd.memset(rt0[:], 0.0)
    nc.gpsimd.affine_select(
        out=rt0[:], in_=rt0[:], compare_op=mybir.AluOpType.not_equal,
        fill=1.0, base=-32, channel_multiplier=1, pattern=[[-1, RH]],
    )
    nc.gpsimd.affine_select(
        out=rt0[:], in_=rt0[:], compare_op=mybir.AluOpType.not_equal,
        fill=1.0, base=32, channel_multiplier=1, pattern=[[-1, RH]],
    )
    rt = pool.tile([RH, RH], BF, tag="rt")
    nc.vector.tensor_copy(rt[:], rt0[:])
    return rt


def build_kernel(T=T_FULL):
    assert T % 512 == 0
    NT512 = T // 512
    NKT = T // P
    EK = E // P
    TS = T // NCORES      # 256

    nc = bacc.Bacc("TRN2", target_bir_lowering=False, debug=False,
                   num_devices=NCORES)

    xTs = nc.dram_tensor("xTs", [E, TS], BF, kind="ExternalInput").ap()
    wqkv = nc.dram_tensor("wqkv", [E, QKH + QL], BF, kind="ExternalInput").ap()
    wqdec = nc.dram_tensor("wqdec", [QL, HPC * QKH], BF, kind="ExternalInput").ap()
    wout = nc.dram_tensor("wout", [HPC * KV, E], BF, kind="ExternalInput").ap()
    cosd = nc.dram_tensor("cosT", [RH, T], F32, kind="ExternalInput").ap()
    sind = nc.dram_tensor("sinT", [RH, T], F32, kind="ExternalInput").ap()
    outT = nc.dram_tensor("outT", [E, T], F32, kind="ExternalOutput").ap()

    xs_r = xTs.rearrange("(ko p) t -> p ko t", p=P)
    wq_r = wqkv.rearrange("(ko p) m -> p ko m", p=P)

    groups = [(i * P, P) for i in range(KV // P)] + [(KV, RH)] + [
        (QKH + i * P, P) for i in range(QL // P)
    ]
    NKVG = KV // P + 1      # groups 0..4 are kv+rope
    NCQ0 = NKVG + 4         # groups 5..8 -> cq half 0

    with tile.TileContext(nc) as tc, ExitStack() as ctx:
        dram = ctx.enter_context(tc.tile_pool(name="dram", bufs=1, space="DRAM"))
        cst = ctx.enter_context(tc.tile_pool(name="cst", bufs=1))
        kvp = ctx.enter_context(tc.tile_pool(name="kvp", bufs=1))
        pp = ctx.enter_context(tc.tile_pool(name="pp", bufs=3, space="PSUM"))
        ppy = ctx.enter_context(tc.tile_pool(name="ppy", bufs=1, space="PSUM"))

        bkv = dram.tile([QKH, TS], BF, tag="bkv")
        bcq0 = dram.tile([QL // 2, TS], BF, tag="bcq0")
        bcq1 = dram.tile([QL // 2, TS], BF, tag="bcq1")
        gkv = dram.tile([NCORES, QKH, TS], BF, tag="gkv")
        gcq0 = dram.tile([NCORES, QL // 2, TS], BF, tag="gcq0")
        gcq1 = dram.tile([NCORES, QL // 2, TS], BF, tag="gcq1")

        # ---- constants (before phase A so gpsimd is free afterwards) ----
        ident0 = cst.tile([P, P], F32, tag="ident0")
        make_identity(nc, ident0[:])
        ident = cst.tile([P, P], BF, tag="ident")
        nc.vector.tensor_copy(ident[:], ident0[:])
        rt = _make_rot64(nc, cst)
        # single triangular mask: mtri[k, q] = 1 iff k <= q (block diagonal)
        mtri = cst.tile([P, P], BF, tag="mtri")
        nc.gpsimd.memset(mtri[:], 1.0)
        nc.gpsimd.affine_select(
            out=mtri[:], in_=mtri[:], compare_op=mybir.AluOpType.is_ge,
            fill=0.0, base=0, channel_multiplier=-1, pattern=[[1, P]],
        )
        ones0 = cst.tile([P, 1], F32, tag="ones0")
        nc.gpsimd.memset(ones0[:], 1.0)
        ones_col = cst.tile([P, 1], BF, tag="ones")
        nc.vector.tensor_copy(ones_col[:], ones0[:])
        # cos/sin tiles declared here; DMAs issued at B/C setup (they are
        # not needed until rope and would delay phase-A weight loads)
        cosT = cst.tile([RH, T], F32, tag="cosT")
        ssinT = cst.tile([RH, T], F32, tag="ssinT")

        ckvT = kvp.tile([P, KV // P, T], BF, tag="ckvT")
        krT = kvp.tile([RH, T], BF, tag="krT")
        krT2 = kvp.tile([RH, T], BF, tag="krT2")
        yT_sb = kvp.tile([P, HPC * KV // P, T], BF, tag="yT_sb")
        wo_sb = kvp.tile([P, HPC * KV // P, E], BF, tag="wo_sb")

        RG = [list(range(NCORES))]

        # ================= Phase A (T-sharded) ============================
        with ExitStack() as actx:
            aw = actx.enter_context(tc.tile_pool(name="aw", bufs=1))
            astp = actx.enter_context(tc.tile_pool(name="astp", bufs=3))

            xs = aw.tile([P, EK, TS], BF, tag="xs")
            nc.scalar.dma_start(xs[:, 0 : EK // 2], xs_r[:, 0 : EK // 2])
            nc.sync.dma_start(xs[:, EK // 2 :], xs_r[:, EK // 2 :])
            # kv-group weights first (they gate the first collective);
            # late cq weights avoid gpsimd so the collectives aren't queued
            # behind transfers
            wqt = [None] * len(groups)
            qeng = [nc.scalar, nc.sync, nc.gpsimd]
            order = list(range(NKVG)) + list(range(NKVG, len(groups)))
            for idx, gi in enumerate(order):
                c0, M = groups[gi]
                eng = (qeng[idx % 3] if gi < NKVG
                       else qeng[idx % 2])
                wa = aw.tile([P, EK, M], BF, tag=f"wqa{gi}", name=f"wqa{gi}")
                eng.dma_start(wa[:], wq_r[:, :, c0 : c0 + M])
                wqt[gi] = wa

            for gi, (c0, M) in enumerate(groups):
                ps = pp.tile([P, TS], F32, tag="mm", name="psA")
                for kc in range(EK):
                    nc.tensor.matmul(
                        ps[:M], wqt[gi][:, kc, :], xs[:, kc, :],
                        start=(kc == 0), stop=(kc == EK - 1),
                    )
                st = astp.tile([P, TS], BF, tag="ast", name="ast")
                nc.vector.tensor_copy(st[:M], ps[:M])
                # bounce writes on gpsimd: they directly precede (and gate)
                # the collectives on that same queue
                if c0 < QKH:
                    nc.gpsimd.dma_start(bkv[c0 : c0 + M, :], st[:M])
                elif c0 < QKH + QL // 2:
                    nc.gpsimd.dma_start(
                        bcq0[c0 - QKH : c0 - QKH + M, :], st[:M])
                else:
                    nc.gpsimd.dma_start(
                        bcq1[c0 - QKH - QL // 2 : c0 - QKH - QL // 2 + M, :],
                        st[:M])
                if gi == NKVG - 1:
                    nc.gpsimd.collective_compute(
                        "AllGather", mybir.AluOpType.bypass,
                        replica_groups=RG, ins=[bkv[:]], outs=[gkv[:]],
                    )
                elif gi == NCQ0 - 1:
                    nc.gpsimd.collective_compute(
                        "AllGather", mybir.AluOpType.bypass,
                        replica_groups=RG, ins=[bcq0[:]], outs=[gcq0[:]],
                    )
            nc.gpsimd.collective_compute(
                "AllGather", mybir.AluOpType.bypass,
                replica_groups=RG, ins=[bcq1[:]], outs=[gcq1[:]],
            )

        # ============ Phases preC + fused B/C =============================
        with ExitStack() as bctx:
            bcp = bctx.enter_context(tc.tile_pool(name="bcp", bufs=1))
            bcs = bctx.enter_context(tc.tile_pool(name="bcs", bufs=2))

            # rope tables + wqd for both heads, prefetched on scalar
            nc.scalar.dma_start(cosT[:], cosd[:])
            nc.scalar.dma_start(ssinT[:], sind[:])
            nc.vector.tensor_scalar_mul(
                ssinT[0 : RH // 2, :], ssinT[0 : RH // 2, :], -1.0
            )
            wqds = []
            for h in range(HPC):
                wqd = bcp.tile([P, QL // P, QKH], BF, tag=f"wqd{h}",
                               name=f"wqd{h}")
                nc.scalar.dma_start(
                    wqd[:],
                    wqdec.rearrange("(ko p) m -> p ko m", p=P)[
                        :, :, h * QKH : (h + 1) * QKH
                    ],
                )
                wqds.append(wqd)

            # unpack kv gather into SBUF residents
            for seg in range(NCORES):
                tsl = slice(seg * TS, (seg + 1) * TS)
                nc.scalar.dma_start(
                    ckvT[:, :, tsl],
                    gkv[seg, 0:KV, :].rearrange("(ko p) t -> p ko t", p=P),
                )
                nc.sync.dma_start(krT[:, tsl], gkv[seg, KV:QKH, :])

            # rope k_r
            for tcc in range(NT512):
                tsl = slice(tcc * 512, (tcc + 1) * 512)
                pr = pp.tile([RH, 512], F32, tag="mm", name="prk")
                nc.tensor.matmul(pr[:], rt[:], krT[:, tsl],
                                 start=True, stop=True)
                nc.vector.tensor_mul(krT2[:, tsl], krT[:, tsl], cosT[:, tsl])
                rot = bcs.tile([RH, 512], F32, tag="rot", name="rotk")
                nc.vector.tensor_mul(rot[:], pr[:], ssinT[:, tsl])
                nc.vector.tensor_add(krT2[:, tsl], krT2[:, tsl], rot[:])

            # v[t, d] via PE transposes of c_kv^T
            v = bcp.tile([P, NKT, KV], BF, tag="v")
            with tc.tile_pool(name="ptr", bufs=1, space="PSUM") as ptr:
                for dc in range(KV // P):
                    for tt in range(NKT):
                        pt = ptr.tile([P, P], BF, tag="tr", name="pt")
                        nc.tensor.transpose(
                            pt[:], ckvT[:, dc, tt * P : (tt + 1) * P], ident[:]
                        )
                        nc.vector.tensor_copy(
                            v[:, tt, dc * P : (dc + 1) * P], pt[:]
                        )

            # c_q windows resident: loaded once, shared by both heads
            cqw = []
            for i4 in range(NT512):
                cqt = bcp.tile([P, QL // P, 2, TS], BF, tag=f"cqw{i4}",
                               name=f"cqw{i4}")
                for s in range(2):
                    nc.scalar.dma_start(
                        cqt[:, 0 : QL // (2 * P), s],
                        gcq0[2 * i4 + s].rearrange("(ko p) t -> p ko t", p=P),
                    )
                    nc.sync.dma_start(
                        cqt[:, QL // (2 * P) : QL // P, s],
                        gcq1[2 * i4 + s].rearrange("(ko p) t -> p ko t", p=P),
                    )
                cqw.append(cqt)

            # W_out resident for phase D, queued behind everything on sync
            wo_r = wout.rearrange("(ko p) e -> p ko e", p=P)
            nc.sync.dma_start(wo_sb[:], wo_r[:])

            pden = bctx.enter_context(
                tc.tile_pool(name="pden", bufs=1, space="PSUM")
            )
            dst = bctx.enter_context(tc.tile_pool(name="dst", bufs=1))

            DK = HPC * KV // P
            wr_eng = [nc.scalar, nc.sync, nc.gpsimd]

            def emit_d_block(tcc):
                # phase-D tile for 512 output columns: interleaved into the
                # last head's loop so its (dependency-free) matmuls fill
                # B/C pipeline bubbles and only the final block runs after
                tsl = slice(tcc * 512, (tcc + 1) * 512)
                for mc in range(E // P):
                    psD = ppy.tile([P, 512], F32, tag=f"y{mc % 4}",
                                   name=f"psD{mc % 4}")
                    for kc in range(DK):
                        nc.tensor.matmul(
                            psD[:], wo_sb[:, kc, mc * P : (mc + 1) * P],
                            yT_sb[:, kc, tsl],
                            start=(kc == 0), stop=(kc == DK - 1),
                        )
                    ost = dst.tile([P, 512], F32, tag=f"ost{mc % 3}",
                                   name="ost")
                    if mc % 2 == 0:
                        nc.vector.tensor_copy(ost[:], psD[:])
                    else:
                        nc.scalar.copy(ost[:], psD[:])
                    wr_eng[mc % 3].dma_start(
                        outT[mc * P : (mc + 1) * P, tsl], ost[:]
                    )

            qgroups = [(KV, RH)] + [(i * P, P) for i in range(KV // P)]
            for h in range(HPC):
                wqd = wqds[h]
                for i4 in range(NT512):
                    qsl = slice(i4 * 512, (i4 + 1) * 512)
                    # ---- B: q^T for queries i4 (SCALE folded in) ----
                    cq = cqw[i4]
                    qTc = [bcs.tile([P, 512], BF, tag=f"qTc{i}",
                                    name=f"qTc{i}") for i in range(KV // P)]
                    qrRaw = bcs.tile([RH, 512], BF, tag="qrRaw", name="qrRaw")
                    qrT = bcs.tile([RH, 512], BF, tag="qrT", name="qrT")
                    for (m0, M) in qgroups:
                        ps = pp.tile([P, 512], F32, tag="mm", name="psB")
                        for kc in range(QL // P):
                            nc.tensor.matmul(
                                ps[:M], wqd[:, kc, m0 : m0 + M],
                                cq[:, kc],
                                start=(kc == 0), stop=(kc == QL // P - 1),
                            )
                        if m0 < KV:
                            nc.vector.tensor_scalar_mul(
                                qTc[m0 // P][:], ps[:], SCALE
                            )
                        else:
                            nc.vector.tensor_scalar_mul(qrRaw[:], ps[:RH], SCALE)
                            pr = pp.tile([RH, 512], F32, tag="mm", name="prq")
                            nc.tensor.matmul(pr[:], rt[:], qrRaw[:],
                                             start=True, stop=True)
                            nc.vector.tensor_mul(qrT[:], qrRaw[:], cosT[:, qsl])
                            rot = bcs.tile([RH, 512], F32, tag="rot",
                                           name="rotq")
                            nc.vector.tensor_mul(rot[:], pr[:], ssinT[:, qsl])
                            nc.vector.tensor_add(qrT[:], qrT[:], rot[:])

                    # ---- C: causal attention for queries i4 ----
                    nj = 4 * i4 + 4
                    psden = pden.tile([1, 512], F32, tag="den", name="psden")
                    psy = [ppy.tile([P, 512], F32, tag=f"y{dc}",
                                    name=f"psy{dc}")
                           for dc in range(KV // P)]
                    for j in range(nj):
                        ksl = slice(j * P, (j + 1) * P)
                        # diagonal trim: key block j only matters for
                        # queries >= 128*r (r = block index within chunk)
                        r = j - 4 * i4
                        qo = P * r if r >= 0 else 0
                        W = 512 - qo
                        ps = pp.tile([P, 512], F32, tag="mm", name="psS")
                        for dc in range(KV // P):
                            nc.tensor.matmul(
                                ps[:, :W], ckvT[:, dc, ksl],
                                qTc[dc][:, qo:],
                                start=(dc == 0), stop=False,
                            )
                        nc.tensor.matmul(
                            ps[:, :W], krT2[:, ksl], qrT[:, qo:],
                            start=False, stop=True,
                        )
                        se = bcs.tile([P, 512], BF, tag="se", bufs=3,
                                      name="se")
                        nc.scalar.activation(se[:, :W], ps[:, :W], AF.Exp)
                        if r >= 0:
                            nc.vector.tensor_mul(
                                se[:, 0:P], se[:, 0:P], mtri[:])
                        nc.tensor.matmul(
                            psden[:, qo:], ones_col[:], se[:, :W],
                            start=(j == 0), stop=(j == nj - 1),
                            skip_group_check=True,
                        )
                        for dc in range(KV // P):
                            nc.tensor.matmul(
                                psy[dc][:, qo:],
                                v[:, j, dc * P : (dc + 1) * P],
                                se[:, :W],
                                start=(j == 0), stop=(j == nj - 1),
                                skip_group_check=True,
                            )
                    deninv = bcs.tile([1, 512], F32, tag="deninv",
                                      name="deninv")
                    nc.vector.reciprocal_approx_fast(out=deninv[:],
                                                     in_=psden[:])
                    denb = bcs.tile([P, 512], F32, tag="denb", name="denb")
                    nc.gpsimd.partition_broadcast(denb[:], deninv[:])
                    for dc in range(KV // P):
                        nc.vector.tensor_mul(
                            yT_sb[:, h * (KV // P) + dc, qsl],
                            psy[dc][:], denb[:],
                        )
                    if h == HPC - 1:
                        emit_d_block(i4)

    nc.compile()
    return nc


_NC_CACHE = {}


def _get_nc(T=T_FULL):
    if T not in _NC_CACHE:
        _NC_CACHE[T] = build_kernel(T)
    return _NC_CACHE[T]


def make_in_maps(x, cos, sin, W_qkv, W_qdec, W_out):
    xT = np.ascontiguousarray(np.asarray(x)[0].T)
    cosT = np.ascontiguousarray(np.asarray(cos).T.astype(np.float32))
    sinT = np.ascontiguousarray(np.asarray(sin).T.astype(np.float32))
    wqkv = np.ascontiguousarray(np.asarray(W_qkv)).astype(NPBF)
    W_qdec = np.asarray(W_qdec)
    W_out = np.asarray(W_out)
    T = xT.shape[1]
    TS = T // NCORES
    in_maps = []
    for c in range(NCORES):
        in_maps.append({
            "xTs": np.ascontiguousarray(
                xT[:, c * TS : (c + 1) * TS]).astype(NPBF),
            "wqkv": wqkv,
            "wqdec": np.ascontiguousarray(
                W_qdec[:, c * HPC * QKH : (c + 1) * HPC * QKH]).astype(NPBF),
            "wout": np.ascontiguousarray(
                W_out[c * HPC * KV : (c + 1) * HPC * KV]).astype(NPBF),
            "cosT": cosT,
            "sinT": sinT,
        })
    return in_maps


def kernel(x, cos, sin, W_qkv, W_qdec, W_out, _trace=False, _tmpdir=None):
    T = np.asarray(x).shape[1]
    nc = _get_nc(T)
    in_maps = make_in_maps(x, cos, sin, W_qkv, W_qdec, W_out)
    res = run_bass_kernel_spmd(
        nc, in_maps, core_ids=list(range(NCORES)),
        trace=_trace, tmpdir=_tmpdir,
    )
    out = np.zeros((E, T), np.float32)
    for r in res.results:
        out += r["outT"]
    kernel.last_results = res
    return np.ascontiguousarray(out.T)[None].astype(np.float32)
